# revision 3
# baseline (speedup 1.0000x reference)
"""Trainium2 Bass kernel for nn_DescriptionAware (dense_mlp).

vs v2:
- gathers back to pipelined indirect_dma_start (int32 offsets, no gpsimd
  library load) against an fp8e4m3 word-emb table (300B rows, x16 scaled;
  the 1/16 is folded into the psum->sbuf copies of pd_agg / arg_ws)
- DoubleRow fp8 matmuls for the two big PE stages:
  * aw (arg weighted sums): sense-pairs contracted 256-deep, 150cyc/matmul
  * final logits: d-chunks 0+1 in one [128,2,512] DoubleRow matmul
- tt tiles fp8; W2 fp8 (x16, unscaled at output copy)
"""

import os
import numpy as np

import concourse.bass as bass
import concourse.mybir as mybir
import concourse.tile as tile
from concourse import bacc
from concourse.bass import IndirectOffsetOnAxis
from concourse.bass_utils import run_bass_kernel_spmd
from concourse.tile_rust import add_dep_helper

B, S, H = 4, 256, 768
C = 64
LD = 128
E = 300
NS = 8
LP = 32
LA = 16
V = 50000
DH = 300

NCORES = 8
CH = 32
DCH = [(0, 128), (128, 256), (256, 300)]
EPAD = 320            # gathered-row stride (DR needs 16B-aligned pair steps)
EMB_SCALE = 16.0      # fp8 table pre-scale; undone in pd_agg/aws copies

F32 = mybir.dt.float32
BF16 = mybir.dt.bfloat16
FP8 = mybir.dt.float8e4
I32 = mybir.dt.int32
AL = mybir.AluOpType
AF = mybir.ActivationFunctionType
DR = mybir.MatmulPerfMode.DoubleRow

# cf (f32) column layout
CF_ONES = 0            # [128,128] all ones
CF_B1 = 128            # [1,300] b1 row
CF_SCBA = 428          # [8,1] scol + ba2
CF_ID8 = 429           # [8,8] f32 identity
NCF = 437
# cw (bf16) column layout
CW_WA2B = 0            # [8,768]
CW_LEMBT = 768         # [128,32]
CW_W2 = 800            # [128,96] W2 chunks; chunk2 row44 col0 = b2
CW_W1L = 896           # [128,1200]
CW_W1P = 2096          # [128,1800]
NCW = 3896
# cq (fp8) column layout
CQ_PARG = 0            # [128,512] 32 blocks x 16 cols (8 valid + 8 zero)
CQ_PARP = 512          # [128,32] pd selectors (2 chunks x 8, 16-step for DR)
CQ_W1L8 = 544          # [128,1280] W1l fp8 x16, 4 chunks at 320-col strides
CQ_LEMB8 = 1824        # [128,32] label_emb.T fp8 x16
CQ_ID8 = 1856          # [8,8] fp8 identity
NCQ = 1864
# cbx (bf16) column layout
CB_X = 0
CB_XT = 1536
CB_ID = 3072
NCB = 3200


def build_program():
    nc = bacc.Bacc("TRN2", target_bir_lowering=False, debug=False, num_devices=NCORES,
                   dynamic_dma_scratch_size=65536, num_swdge_queues=2)

    dt = nc.dram_tensor
    t_wemb8 = dt("wemb8", [V, E], FP8, kind="ExternalInput")
    t_ag = dt("ag", [128, 34], I32, kind="ExternalInput")   # 32 arg cols, 2 pd cols
    t_cf = dt("cf", [128, NCF], F32, kind="ExternalInput")
    t_cbx = dt("cbx", [128, NCB], BF16, kind="ExternalInput")
    t_cw = dt("cw", [128, NCW], BF16, kind="ExternalInput")
    t_cq = dt("cq", [128, NCQ], FP8, kind="ExternalInput")
    t_wa1a = dt("wa1a", [128, 6 * H], BF16, kind="ExternalInput")
    t_wa1b = dt("wa1b", [128, 3 * H], BF16, kind="ExternalInput")
    t_smk = dt("smk", [128, 2], BF16, kind="ExternalInput")
    t_w1x = dt("w1x", [128, 6 * DH], BF16, kind="ExternalInput")
    t_out = dt("out", [16, 512], F32, kind="ExternalOutput")

    with tile.TileContext(nc) as tc:
        with tc.tile_pool(name="sb", bufs=1) as sb, \
             tc.tile_pool(name="sbt", bufs=4) as sbt, \
             tc.tile_pool(name="ppw", bufs=3, space="PSUM") as ppw, \
             tc.tile_pool(name="ppa", bufs=2, space="PSUM") as ppa, \
             tc.tile_pool(name="ppo", bufs=1, space="PSUM") as ppo:

            # ---------------- DMAs ----------------
            # sync queue: offsets first, then x, wa1a
            ag = sb.tile([128, 34], I32, tag="ag")
            nc.sync.dma_start(out=ag[:], in_=t_ag[:])
            smk = sb.tile([128, 2], BF16, tag="smk")
            nc.sync.dma_start(out=smk[:], in_=t_smk[:])
            cbx = sb.tile([128, NCB], BF16, tag="cbx")
            nc.sync.dma_start(out=cbx[:], in_=t_cbx[:])
            wa1a = sb.tile([128, 6 * H], BF16, tag="wa1a")
            nc.sync.dma_start(out=wa1a[:], in_=t_wa1a[:])

            # scalar queue
            cf = sb.tile([128, NCF], F32, tag="cf")
            nc.scalar.dma_start(out=cf[:], in_=t_cf[:])
            cq = sb.tile([128, NCQ], FP8, tag="cq")
            nc.scalar.dma_start(out=cq[:], in_=t_cq[:])
            w1x = sb.tile([128, 6 * DH], BF16, tag="w1x")
            nc.scalar.dma_start(out=w1x[:], in_=t_w1x[:])
            wa1b = sb.tile([128, 3 * H], BF16, tag="wa1b")
            nc.scalar.dma_start(out=wa1b[:], in_=t_wa1b[:])
            cw = sb.tile([128, NCW], BF16, tag="cw")
            nc.scalar.dma_start(out=cw[:], in_=t_cw[:])

            # gpsimd: pipelined indirect gathers (pd first, then per-cb args)
            pdG = sb.tile([128, 2 * EPAD], FP8, tag="pdG")
            gq = 0
            for h in range(2):
                gi = nc.gpsimd.indirect_dma_start(
                    out=pdG[:, EPAD * h:EPAD * h + E], out_offset=None, in_=t_wemb8[:],
                    in_offset=IndirectOffsetOnAxis(ap=ag[:, 32 + h:33 + h], axis=0))
                gi.ins.queue = f"qPoolDynamic{gq or ''}"
                gi.ins.single_packet = True
                gq ^= 1
            argG = []
            for cb in range(4):
                g_ = sb.tile([128, 8 * EPAD], FP8, tag=f"argG{cb}")
                for j in range(8):
                    gi = nc.gpsimd.indirect_dma_start(
                        out=g_[:, EPAD * j:EPAD * j + E], out_offset=None, in_=t_wemb8[:],
                        in_offset=IndirectOffsetOnAxis(ap=ag[:, 8 * cb + j:8 * cb + j + 1],
                                                       axis=0))
                    gi.ins.queue = f"qPoolDynamic{gq or ''}"
                    gi.ins.single_packet = True
                    gq ^= 1
                argG.append(g_)

            ident = cbx[:, CB_ID:CB_ID + 128]
            xs = cbx[:, CB_X:CB_X + 1536]
            xT = cbx[:, CB_XT:CB_XT + 1536]
            wa2b = cw[0:8, CW_WA2B:CW_WA2B + H]
            lembT = cw[:, CW_LEMBT:CW_LEMBT + 32]
            w1l = [cw[0:(44 if i == 3 else 128), CW_W1L + DH * i:CW_W1L + DH * (i + 1)]
                   for i in range(4)]
            w1p = [cw[:, CW_W1P + DH * i:CW_W1P + DH * (i + 1)] for i in range(6)]
            wa1 = [wa1a[:, H * i:H * (i + 1)] for i in range(6)] + \
                  [wa1b[:, H * 0:H * 1], wa1b[:, H * 1:H * 2], wa1b[0:45, H * 2:H * 3]]
            w2c = [cw[0:(45 if i == 2 else 128), CW_W2 + 32 * i:CW_W2 + 32 * (i + 1)]
                   for i in range(3)]
            parpv = cq[:, CQ_PARP:CQ_PARP + 32].rearrange("p (two m) -> p two m", two=2)

            # ---------------- pred span row + predT ----------------
            prow_h = [ppw.tile([1, 384], F32, tag="w", name=f"prow{nb}") for nb in range(2)]
            for nb in range(2):
                for st in range(2):
                    nc.tensor.matmul(out=prow_h[nb][:], lhsT=smk[:, st:st + 1],
                                     rhs=xs[:, 768 * st + 384 * nb:768 * st + 384 * (nb + 1)],
                                     start=(st == 0), stop=(st == 1), tile_position=(0, 0))
            prow_s = sb.tile([1, H], BF16, tag="prow_s")
            for nb in range(2):
                nc.scalar.copy(out=prow_s[0:1, 384 * nb:384 * (nb + 1)], in_=prow_h[nb][:])
            predT = []
            for hc in range(6):
                tp = ppw.tile([128, 1], BF16, tag="w", name=f"tpp{hc}")
                nc.tensor.transpose(out=tp[:], in_=prow_s[0:1, 128 * hc:128 * (hc + 1)],
                                    identity=ident[0:1, 0:1])
                pt = sb.tile([128, 1], BF16, tag=f"predT{hc}")
                nc.vector.tensor_copy(out=pt[:], in_=tp[:])
                predT.append(pt)

            attk = []
            for k in range(6):
                a_ = sb.tile([128, 8], BF16, tag=f"attk{k}")
                nc.vector.tensor_copy(out=a_[:], in_=predT[k][:, 0:1].to_broadcast([128, 8]))
                attk.append(a_)

            # ---------------- hxT ----------------
            hxT = []
            for dc, (d0, d1) in enumerate(DCH):
                ds_ = d1 - d0
                hp_ = ppw.tile([ds_, S], F32, tag="w", name=f"hxp{dc}")
                for hc in range(6):
                    nc.tensor.matmul(out=hp_[:], lhsT=w1x[:, DH * hc + d0:DH * hc + d1],
                                     rhs=xT[:, 256 * hc:256 * (hc + 1)],
                                     start=(hc == 0), stop=(hc == 5))
                hs = sb.tile([ds_, S], BF16, tag=f"hxT{dc}")
                nc.scalar.copy(out=hs[:], in_=hp_[:])
                hxT.append(hs)

            # ---------------- hp row -> hpbT ----------------
            hprow = ppw.tile([1, DH], F32, tag="w", name="hprow")
            for i in range(6):
                nc.tensor.matmul(out=hprow[:], lhsT=predT[i][:], rhs=w1p[i][:],
                                 start=(i == 0), stop=(i == 5), tile_position=(0, 0))
            hpb = sb.tile([1, DH], BF16, tag="hpb")
            nc.vector.tensor_tensor(out=hpb[:], in0=hprow[:], in1=cf[0:1, CF_B1:CF_B1 + DH],
                                    op=AL.add)
            hpbT = []
            for dc, (d0, d1) in enumerate(DCH):
                tp2 = ppw.tile([d1 - d0, 1], BF16, tag="w", name=f"tp2{dc}")
                nc.tensor.transpose(out=tp2[:], in_=hpb[0:1, d0:d1], identity=ident[0:1, 0:1])
                hb = sb.tile([d1 - d0, 1], F32, tag=f"hpbT{dc}")
                nc.vector.tensor_copy(out=hb[:], in_=tp2[:])
                hpbT.append(hb)

            # ---------------- pd_agg + attk 6-8 ----------------
            pdps = ppw.tile([16, E], F32, tag="w", name="pdps")
            nc.tensor.matmul(out=pdps[:], lhsT=parpv,
                             rhs=pdG[:].rearrange("p (two n) -> p two n", two=2)[:, :, 0:E],
                             start=True, stop=True, perf_mode=DR)
            pd_agg = sb.tile([8, E], BF16, tag="pd_agg")
            nc.vector.tensor_scalar(out=pd_agg[:], in0=pdps[0:8, 0:E],
                                    scalar1=1.0 / EMB_SCALE, scalar2=None, op0=AL.mult)
            for e in range(2):
                tp = ppw.tile([128, 8], BF16, tag="w", name=f"tpa{e}")
                nc.tensor.transpose(out=tp[:], in_=pd_agg[:, 128 * e:128 * (e + 1)],
                                    identity=ident[0:8, 0:8])
                a_ = sb.tile([128, 8], BF16, tag=f"attk{6 + e}")
                nc.vector.tensor_copy(out=a_[:], in_=tp[:])
                attk.append(a_)
            tp = ppw.tile([44, 8], BF16, tag="w", name="tpb")
            nc.tensor.transpose(out=tp[:], in_=pd_agg[:, 256:300], identity=ident[0:8, 0:8])
            a_ = sb.tile([45, 8], BF16, tag="attk8")
            nc.vector.memset(a_[:, :], 1.0)
            nc.vector.tensor_copy(out=a_[0:44, :], in_=tp[:])
            attk.append(a_)

            # ---------------- attention MLP -> wcolx ----------------
            hidp = []
            for nb in range(2):
                hp2 = ppw.tile([8, 384], F32, tag="w", name=f"hid{nb}")
                for k in range(9):
                    nc.tensor.matmul(out=hp2[:], lhsT=attk[k][:],
                                     rhs=wa1[k][:, 384 * nb:384 * (nb + 1)],
                                     start=(k == 0), stop=(k == 8))
                hidp.append(hp2)
            hid = sb.tile([8, H], F32, tag="hid")
            for nb in range(2):
                nc.scalar.activation(out=hid[:, 384 * nb:384 * (nb + 1)], in_=hidp[nb][:],
                                     func=AF.Relu)
            scr = sb.tile([8, H], F32, tag="scr")
            nc.vector.tensor_tensor(out=scr[:], in0=hid[:], in1=wa2b[:, :], op=AL.mult)
            wraw = sb.tile([8, 1], F32, tag="wraw")
            nc.vector.tensor_reduce(out=wraw[:], in_=scr[:], axis=mybir.AxisListType.X,
                                    op=AL.add)
            wsb = sb.tile([8, 1], F32, tag="wsb")
            nc.vector.tensor_scalar(out=wsb[:], in0=wraw[:],
                                    scalar1=cf[0:8, CF_SCBA:CF_SCBA + 1], scalar2=None,
                                    op0=AL.add)
            wsbT = ppw.tile([1, 8], F32, tag="w", name="wsbT")
            nc.tensor.transpose(out=wsbT[:], in_=wsb[:], identity=cf[0:8, CF_ID8:CF_ID8 + 8])
            expr = sb.tile([1, 8], F32, tag="expr")
            nc.scalar.activation(out=expr[:], in_=wsbT[:], func=AF.Exp)
            sumr = sb.tile([1, 1], F32, tag="sumr")
            nc.vector.tensor_reduce(out=sumr[:], in_=expr[:], axis=mybir.AxisListType.X,
                                    op=AL.add)
            rsr = sb.tile([1, 1], F32, tag="rsr")
            nc.vector.reciprocal(out=rsr[:], in_=sumr[:])
            wrow = sb.tile([1, 8], F32, tag="wrow")
            nc.vector.tensor_scalar(out=wrow[:], in0=expr[:], scalar1=rsr[:], scalar2=None,
                                    op0=AL.mult)
            wcps = ppw.tile([128, 8], F32, tag="w", name="wcps")
            nc.tensor.matmul(out=wcps[:], lhsT=cf[0:1, CF_ONES:CF_ONES + 128], rhs=wrow[:],
                             start=True, stop=True)
            wcolx = sb.tile([128, 8], F32, tag="wcolx")
            nc.vector.tensor_copy(out=wcolx[:], in_=wcps[:])

            # wsel tiles: one [128,16] fp8 per sense-pair (2 ops each)
            wsel = []
            for cb in range(4):
                for jp in range(4):
                    w_ = sbt.tile([128, 32], FP8, tag="wselp", name=f"wsel{cb}_{jp}")
                    for s in range(2):
                        j = 2 * jp + s
                        i = 8 * cb + j
                        nc.vector.tensor_scalar(
                            out=w_[:, 16 * s:16 * s + 16],
                            in0=cq[:, CQ_PARG + 16 * i:CQ_PARG + 16 * (i + 1)],
                            scalar1=wcolx[:, j:j + 1], scalar2=None, op0=AL.mult)
                    wsel.append(w_)

            pb_init = None
            tt44 = []
            for i in range(4):
                t_ = sb.tile([45, 512], BF16, tag=f"tt44_{i}")
                nc.vector.memset(t_[:, :], 1.0)
                tt44.append(t_)

            # ---------------- per class-block ----------------
            prev_group_end = {}
            outp2 = None
            for cb in range(4):
                cyc = cb // 2
                if cb % 2 == 0:
                    outp2 = [ppo.tile([128, 512], F32, tag=f"out{h}", name=f"outp{h}_{cyc}")
                             for h in range(2)]
                    prev_group_end = {(h, r): None for h in range(2) for r in (0, 32, 64, 96)}
                aw = ppa.tile([16, E], F32, tag="acc", name=f"aw{cb}")
                for jp in range(4):
                    nc.tensor.matmul(
                        out=aw[:],
                        lhsT=wsel[4 * cb + jp][:].rearrange("p (two m) -> p two m", two=2),
                        rhs=argG[cb][:, 640 * jp:640 * (jp + 1)].rearrange(
                            "p (two n) -> p two n", two=2)[:, :, 0:E],
                        start=(jp == 0), stop=(jp == 3), perf_mode=DR)
                aws = sbt.tile([8, E], BF16, tag="aws")
                nc.vector.tensor_copy(out=aws[:], in_=aw[0:8, 0:E])

                # hl lhsT pairs: A = (lembT8 | liTa0), B = (liTa1 | liTa2)
                pairA = sbt.tile([128, 32], FP8, tag="pairA")
                pairB = sbt.tile([128, 32], FP8, tag="pairB")
                nc.vector.memset(pairA[:, :], 0.0)
                nc.vector.memset(pairB[:, :], 0.0)
                nc.vector.tensor_copy(out=pairA[:, 0:8],
                                      in_=cq[:, CQ_LEMB8 + 8 * cb:CQ_LEMB8 + 8 * (cb + 1)])
                dsts = [(pairA, 16), (pairB, 0), (pairB, 16)]
                for e, (e0, e1) in enumerate(DCH):
                    tp3 = ppw.tile([e1 - e0, 8], BF16, tag="w", name=f"tp3{e}")
                    nc.tensor.transpose(out=tp3[:], in_=aws[:, e0:e1], identity=ident[0:8, 0:8])
                    dt_, c0 = dsts[e]
                    nc.vector.tensor_copy(out=dt_[0:e1 - e0, c0:c0 + 8], in_=tp3[:])

                hl = ppw.tile([16, DH], F32, tag="w", name=f"hl{cb}")
                for ip, pr in enumerate((pairA, pairB)):
                    nc.tensor.matmul(
                        out=hl[:],
                        lhsT=pr[:].rearrange("p (two m) -> p two m", two=2),
                        rhs=cq[:, CQ_W1L8 + 640 * ip:CQ_W1L8 + 640 * (ip + 1)].rearrange(
                            "p (two n) -> p two n", two=2)[:, :, 0:E],
                        start=(ip == 0), stop=(ip == 1), perf_mode=DR)
                hls = sbt.tile([8, DH], BF16, tag="hls")
                nc.vector.tensor_scalar(out=hls[:], in0=hl[0:8, 0:E],
                                        scalar1=1.0 / (EMB_SCALE * EMB_SCALE),
                                        scalar2=None, op0=AL.mult)

                biasT = []
                for dc, (d0, d1) in enumerate(DCH):
                    tp4 = ppw.tile([d1 - d0, 8], BF16, tag="w", name=f"tp4{dc}")
                    nc.tensor.transpose(out=tp4[:], in_=hls[:, d0:d1], identity=ident[0:8, 0:8])
                    bt = sbt.tile([d1 - d0, 8], F32, tag=f"biasT{dc}", name=f"bt{dc}")
                    nc.vector.tensor_scalar(out=bt[:], in0=tp4[:], scalar1=hpbT[dc][:],
                                            scalar2=None, op0=AL.add)
                    biasT.append(bt)

                ttg = {}
                for cl in range(4):
                    for dc in range(2):
                        tt = sbt.tile([128, 512], BF16, tag="t", bufs=8, name=f"tt{cl}_{dc}")
                        for half in range(2):
                            nc.vector.tensor_scalar(
                                out=tt[:, 256 * half:256 * (half + 1)], in0=hxT[dc][:],
                                scalar1=biasT[dc][:, 2 * cl + half:2 * cl + half + 1],
                                scalar2=0.0, op0=AL.add, op1=AL.max)
                        ttg[(cl, dc)] = tt
                    t44 = tt44[cl]
                    for half in range(2):
                        nc.scalar.activation(
                            out=t44[0:44, 256 * half:256 * (half + 1)], in_=hxT[2][:],
                            func=AF.Relu,
                            bias=biasT[2][:, 2 * cl + half:2 * cl + half + 1])
                    ttg[(cl, 2)] = t44
                for dc in range(3):
                    ds_ = 45 if dc == 2 else 128
                    for cl in (0, 2, 1, 3):
                        cp = 4 * cb + cl
                        h = cp % 2
                        row = 32 * ((cp % 8) // 2)
                        mm = nc.tensor.matmul(out=outp2[h][row:row + 32, :], lhsT=w2c[dc][:],
                                              rhs=ttg[(cl, dc)][0:ds_, :], start=(dc == 0),
                                              stop=(dc == 2), tile_position=(0, row),
                                              skip_group_check=True)
                        if dc == 0 and prev_group_end[(h, row)] is not None:
                            add_dep_helper(mm.ins, prev_group_end[(h, row)], sync=False,
                                           reason="serialize psum accumulation groups per bank")
                        if dc == 2:
                            prev_group_end[(h, row)] = mm.ins

                if cb % 2 == 1:
                    for h in range(2):
                        osb = sb.tile([128, 512], F32, tag=f"osb{cyc}{h}", name=f"osb{cyc}{h}")
                        nc.vector.tensor_copy(out=osb[:], in_=outp2[h][:])
                        nc.sync.dma_start(out=t_out[8 * cyc + h:8 * cyc + 8:2, :],
                                          in_=osb[0:128:32, :])

    nc.compile()
    return nc


def _pack(a, rows, cols):
    k = rows // 128
    return np.ascontiguousarray(a.reshape(k, 128, cols).transpose(1, 0, 2).reshape(128, k * cols))


def make_in_maps(inputs):
    import ml_dtypes
    bf = ml_dtypes.bfloat16
    f8 = ml_dtypes.float8_e4m3fn
    x = np.asarray(inputs["x"], np.float32)
    pred_start = np.asarray(inputs["pred_start"]).astype(np.int64)
    pred_end = np.asarray(inputs["pred_end"]).astype(np.int64)
    pdi = np.asarray(inputs["pred_desc_ids"]).astype(np.int64)
    adi = np.asarray(inputs["arg_desc_ids"]).astype(np.int64)
    label_emb = np.asarray(inputs["label_emb"], np.float32)
    word_emb = np.asarray(inputs["word_emb"], np.float32)
    Wa1 = np.asarray(inputs["Wa1"], np.float32)
    ba1 = np.asarray(inputs["ba1"], np.float32)
    Wa2 = np.asarray(inputs["Wa2"], np.float32)
    ba2 = np.asarray(inputs["ba2"], np.float32)
    W1 = np.ascontiguousarray(np.asarray(inputs["W1"], np.float32))
    b1 = np.asarray(inputs["b1"], np.float32)
    W2c = np.asarray(inputs["W2"], np.float32).reshape(DH)
    b2 = np.asarray(inputs["b2"], np.float32)

    wemb8 = np.ascontiguousarray((word_emb * EMB_SCALE).astype(f8))

    wa1_aug = np.zeros((1152, H), np.float32)
    wa1_aug[:1068] = Wa1
    wa1_aug[1068] = ba1
    wa1_p = _pack(wa1_aug, 1152, H).astype(bf)
    wa1a = np.ascontiguousarray(wa1_p[:, 0:6 * H])
    wa1b = np.ascontiguousarray(wa1_p[:, 6 * H:9 * H])
    w1x_p = _pack(W1[0:768], 768, DH).astype(bf)
    w1l_p = np.zeros((128, 4 * DH), bf)
    w1l_p[:, 0:DH] = W1[768:896].astype(bf)
    w1l_p[:, DH:2 * DH] = W1[896:1024].astype(bf)
    w1l_p[:, 2 * DH:3 * DH] = W1[1024:1152].astype(bf)
    w1l_p[0:44, 3 * DH:4 * DH] = W1[1152:1196].astype(bf)
    w1p_p = _pack(np.ascontiguousarray(W1[1196:1964]), 768, DH).astype(bf)

    w2_p = np.zeros((128, 96), bf)
    w2_p[0:128, 0] = W2c[0:128].astype(bf)
    w2_p[0:128, 32] = W2c[128:256].astype(bf)
    w2_p[0:44, 64] = W2c[256:300].astype(bf)
    w2_p[44, 64] = bf(float(b2[0]))

    cf = np.zeros((128, NCF), np.float32)
    cf[:, CF_ONES:CF_ONES + 128] = 1.0
    cf[0, CF_B1:CF_B1 + DH] = b1
    cf[0:8, CF_ID8:CF_ID8 + 8] = np.eye(8, dtype=np.float32)

    cbx_common = np.zeros((128, NCB), bf)
    cbx_common[:, CB_ID:CB_ID + 128] = np.eye(128, dtype=np.float32).astype(bf)

    p = np.arange(128)

    in_maps = []
    for core in range(NCORES):
        b, ch = core // 2, core % 2
        ids_a = adi[b, :, ch * CH:(ch + 1) * CH, :]          # [8, 32, 16]
        ids_p = pdi[b]                                       # [8, 32]
        alen = np.maximum(1, (ids_a > 0).sum(-1)).astype(np.float32)
        plen = np.maximum(1, (ids_p > 0).sum(-1)).astype(np.float32)
        pmask0 = (ids_p > 0).any(-1)
        scol = np.where(pmask0, 0.0, -100000.0).astype(np.float32)

        # gather offsets: col i = 8cb+j -> rows (p) = token (c=8cb+p//16, l=p%16)
        ag = np.zeros((128, 34), np.int32)
        for cb in range(4):
            for j in range(8):
                ag[:, 8 * cb + j] = ids_a[j, 8 * cb + p // 16, p % 16]
        for h in range(2):
            ag[:, 32 + h] = ids_p[4 * h + p // 32, p % 32]

        # parg: [128, 32*8] fp8: delta(class) * mask / len
        parg = np.zeros((128, 512), np.float32)
        for cb in range(4):
            for j in range(8):
                i = 8 * cb + j
                cwi = p // 16
                ids_row = ids_a[j, 8 * cb + cwi, p % 16]
                valid = ids_row > 0
                rl = valid / alen[j, 8 * cb + cwi]
                for m in range(8):
                    parg[:, 16 * i + m] = (cwi == m) * rl
        parp = np.zeros((128, 32), np.float32)
        for h in range(2):
            n_ = 4 * h + p // 32
            ids_row = ids_p[n_, p % 32]
            rl = (ids_row > 0) / plen[n_]
            for m in range(8):
                parp[:, 16 * h + m] = (n_ == m) * rl

        cfc = cf.copy()
        cfc[0:8, CF_SCBA] = scol + float(ba2[0])

        cq_ = np.zeros((128, NCQ), f8)
        cq_[:, CQ_PARG:CQ_PARG + 512] = parg.astype(f8)
        cq_[:, CQ_PARP:CQ_PARP + 32] = parp.astype(f8)
        for i in range(4):
            r0, r1 = [(768, 896), (896, 1024), (1024, 1152), (1152, 1196)][i]
            cq_[0:r1 - r0, CQ_W1L8 + 320 * i:CQ_W1L8 + 320 * i + DH] = \
                (W1[r0:r1] * EMB_SCALE).astype(f8)
        cq_[:, CQ_LEMB8:CQ_LEMB8 + 32] = \
            (label_emb[ch * CH:(ch + 1) * CH, :].T * EMB_SCALE).astype(f8)
        cq_[0:8, CQ_ID8:CQ_ID8 + 8] = np.eye(8, dtype=np.float32).astype(f8)

        cw_ = np.zeros((128, NCW), bf)
        cw_[0:8, CW_WA2B:CW_WA2B + H] = np.broadcast_to(Wa2.reshape(1, H), (8, H)).astype(bf)
        cw_[:, CW_LEMBT:CW_LEMBT + 32] = label_emb[ch * CH:(ch + 1) * CH, :].T.astype(bf)
        cw_[:, CW_W2:CW_W2 + 96] = w2_p
        cw_[:, CW_W1L:CW_W1L + 4 * DH] = w1l_p
        cw_[:, CW_W1P:CW_W1P + 6 * DH] = w1p_p

        cbx = cbx_common.copy()
        xb = x[b]
        cbx[:, CB_X:CB_X + 1536] = xb.reshape(2, 128, H).transpose(1, 0, 2).reshape(128, 1536).astype(bf)
        cbx[:, CB_XT:CB_XT + 1536] = xb.T.reshape(6, 128, 256).transpose(1, 0, 2).reshape(128, 1536).astype(bf)

        ps_, pe_ = int(pred_start[b]), int(pred_end[b])
        rspl = 1.0 / max(1, pe_ - ps_)
        smk = np.zeros((128, 2), np.float32)
        for st in range(2):
            pos = 128 * st + p
            smk[:, st] = ((pos >= ps_) & (pos < pe_)) * rspl

        in_maps.append({
            "wemb8": wemb8,
            "ag": ag,
            "cf": cfc,
            "cbx": cbx,
            "cw": cw_,
            "cq": cq_,
            "wa1a": wa1a,
            "wa1b": wa1b,
            "smk": smk.astype(bf),
            "w1x": w1x_p,
        })
    return in_maps


def assemble(results):
    logits = np.empty((B, S, C), np.float32)
    for core in range(NCORES):
        b, ch = core // 2, core % 2
        r = results[core]["out"].reshape(CH, S)
        logits[b, :, ch * CH:(ch + 1) * CH] = r.T
    return logits


_NC_CACHE = {}
LAST_RESULTS = None


def kernel(**inputs):
    global LAST_RESULTS
    if "nc" not in _NC_CACHE:
        _NC_CACHE["nc"] = build_program()
    nc = _NC_CACHE["nc"]
    in_maps = make_in_maps(inputs)
    trace = bool(os.environ.get("KBENCH_TRACE"))
    res = run_bass_kernel_spmd(nc, in_maps, core_ids=list(range(NCORES)), trace=trace)
    LAST_RESULTS = res
    return assemble(res.results)
